# revision 12
# baseline (speedup 1.0000x reference)
"""Trainium2 Bass kernel for nn_AttentionBlock (B=4, C=H=W=S=256). v5

reference:
  q = Wq @ query + bq   (1x1 conv over channel dim)
  k = Wk @ key_in + bk
  v = Wv @ value + bv
  scores[b,i,h,w] = sum_j q[b,i,h,j] * k[b,j,i,w]
  attn = softmax(scores, -1)
  out[b,i,h,w] = sum_j attn[b,i,h,j] * v[b,i,j,w]
  return sigmoid(out)

Sharding: 8 cores = (b, g) with b=core//2, g=core%2; each core computes
out[b, g*128:(g+1)*128, :, :].

v5 design (vs v4 baseline, 618us measured on HW):
  - q conv computed DIRECTLY TRANSPOSED: per (h, jb) the query tile
    [c, j] is the stationary operand and WqT [c, i] the moving one, so
    the PE emits qT[j, i] straight into PSUM. Kills the 512 identity
    transposes (~27us PE) and the 176us of DVE COPY + scalar copyback
    drains that v4 spent rotating the conv output.
  - bias algebra (validated exact in fp64):
      * bk drops entirely: its score contribution is constant along the
        softmax axis w, and softmax is shift-invariant.
      * bq folds into the scores matmul: qT's 257th h-column holds
        bq[i]; the matmul then emits bq[i]*colsum_k0[i,w] in psc col
        256, which is exactly the per-partition (w) bias exp needs.
      * bv folds into the tanh bias: sum_j attn = 1, so attn@(v0+bv) =
        attn@v0 + bv[i]; tanh arg gets + bv[i]/2 via a broadcast tile.
      * softmax normalization + sigmoid run as ONE activation per
        (io,hb): ob = tanh(po * rs + bv[i]/2) with scale=rs=0.5/rowsum
        (rowsum from vt's 2.0-column), then a Pool-engine *0.5+0.5
        affine per chunk. No yt staging tile, no batched-tanh pass.
  - phase order V -> Q -> B so the v readback DMA (chunked, i-major)
    can stream during phase Q; k input chunks prefetch 3 deep, v
    readbacks 2 deep (just-in-time: V and Q windows are DMA-bound, so
    deeper prefetch only steals their bandwidth).
  - drains split DVE/ACT; DMA issue spread over SP (loads), ACT
    (stores), DVE (v readback), Pool (weights) queues.
  => per-core HBM traffic unchanged (~134MB: q/k/v inputs f16 84MB,
    v round-trip bf16 33.5MB, out f16 16.8MB) but phases V/Q run as
    pure DMA races while phase B is ACT-bound with its DMA hidden.
"""

import numpy as np

import concourse.bass as bass
import concourse.tile as tile
from concourse import bacc, mybir
from concourse.bass_utils import run_bass_kernel_spmd

C = 256
HALF = 128          # output channels per core
N_CORES = 8
ICHUNK = 4          # i values per phase-B chunk
A_CHUNK = 1024      # flattened spatial elems per phase-V chunk
QCH = 4             # h rows per phase-Q chunk

_CACHE = {}


def build_nc(repeat=1):
    key = ("nc", repeat)
    if key in _CACHE:
        return _CACHE[key]
    f32 = mybir.dt.float32
    f16 = mybir.dt.float16
    bf16 = mybir.dt.bfloat16

    nc = bacc.Bacc("TRN2", target_bir_lowering=False, debug=False,
                   num_devices=N_CORES)

    query_b = nc.dram_tensor("query_b", [C, C, C], f16, kind="ExternalInput").ap()
    key_h = nc.dram_tensor("key_h", [C, HALF, C], f16, kind="ExternalInput").ap()
    value_b = nc.dram_tensor("value_b", [C, C, C], f16, kind="ExternalInput").ap()
    wqT = nc.dram_tensor("wqT", [C, HALF], f16, kind="ExternalInput").ap()
    wkT = nc.dram_tensor("wkT", [C, C], f16, kind="ExternalInput").ap()
    wvT = nc.dram_tensor("wvT", [C, HALF], f16, kind="ExternalInput").ap()
    bq_bc = nc.dram_tensor("bq_bc", [128, 2, HALF], f32, kind="ExternalInput").ap()
    bvh_bc = nc.dram_tensor("bvh_bc", [128, HALF], f32, kind="ExternalInput").ap()
    out_b = nc.dram_tensor("out_b", [HALF, C, C], f16, kind="ExternalOutput").ap()

    v_scr = nc.dram_tensor("v_scr", [HALF, C, C], bf16).ap()

    HJ = C * C  # 65536
    KCH = ICHUNK * C  # flattened (il, w) per phase-B chunk

    qv_in = query_b.rearrange("(cb c) h j -> c cb h j", c=128)
    vv_in = value_b.rearrange("(cb c) j w -> c cb (j w)", c=128)
    kv_in = key_h.rearrange("(cb c) il w -> c cb (il w)", c=128)
    v_flat = v_scr.rearrange("i j w -> i (j w)")

    Exp = mybir.ActivationFunctionType.Exp
    Tanh = mybir.ActivationFunctionType.Tanh

    with tile.TileContext(nc) as tc:
        with (
            tc.tile_pool(name="weights", bufs=1) as wpool,
            tc.tile_pool(name="v_in", bufs=2) as v_in_pool,
            tc.tile_pool(name="v_out", bufs=2) as v_out_pool,
            tc.tile_pool(name="q_in", bufs=2) as q_in_pool,
            tc.tile_pool(name="ps512", bufs=2, space="PSUM") as ps512,
            tc.tile_pool(name="b_kin", bufs=4) as b_kin,
            tc.tile_pool(name="b_ksb", bufs=4) as b_ksb,
            tc.tile_pool(name="b_vt", bufs=2) as b_vt,
            tc.tile_pool(name="b_et", bufs=2) as b_et,
            tc.tile_pool(name="b_ob", bufs=2) as b_ob,
            tc.tile_pool(name="b_rs", bufs=8) as b_rs,
            tc.tile_pool(name="b_psc", bufs=2, space="PSUM") as b_psc,
            tc.tile_pool(name="b_po", bufs=2, space="PSUM") as b_po,
        ):
            wq_sb = wpool.tile([128, 2, HALF], f16)
            wk_sb = wpool.tile([128, 2, C], f16)
            wv_sb = wpool.tile([128, 2, HALF], f16)
            nc.gpsimd.dma_start(out=wq_sb, in_=wqT.rearrange("(cb c) i -> c cb i", c=128))
            nc.gpsimd.dma_start(out=wk_sb, in_=wkT.rearrange("(cb c) j -> c cb j", c=128))
            nc.gpsimd.dma_start(out=wv_sb, in_=wvT.rearrange("(cb c) i -> c cb i", c=128))
            bqbc_sb = wpool.tile([128, 2, HALF], f32)
            bvbc_sb = wpool.tile([128, HALF], f32)
            nc.gpsimd.dma_start(out=bqbc_sb, in_=bq_bc)
            nc.gpsimd.dma_start(out=bvbc_sb, in_=bvh_bc)
            # SBUF-resident transposed q for the whole kernel, i-last so
            # the conv drains write contiguous 128-elem runs:
            # qT[j, h, jb, i] = q0[i, h, jb*128+j] + bq[i]  (uniform +bq[i]
            # IS the score bias: sum_j k0*(q0+bq) = scores0 + bq*colsum_k0)
            qT = wpool.tile([128, C, 2, HALF], f16)

            def body(_it=None):
                # ---- Phase 1: V and Q convs interleaved per chunk ----
                # V: natural [i,(j,w)] conv -> DRAM round-trip (ap-512,
                # weights stationary). Q: direct-transposed conv (query
                # tile stationary, ap-128) -> resident qT. Interleaving
                # keeps the PE denser (HAM throttle) and both input
                # streams + the v_scr writeback on the DMA rings at once.
                # V runs 2 chunks per t for t<32 so v_scr completes at the
                # halfway point and phase-B v readbacks stream during the
                # entire second half of phase 1.
                def v_chunk(t):
                    sl = slice(t * A_CHUNK, (t + 1) * A_CHUNK)
                    vc = v_in_pool.tile([128, 2, A_CHUNK], f16, tag="vc")
                    nc.sync.dma_start(out=vc, in_=vv_in[:, :, sl])
                    vs = v_out_pool.tile([128, A_CHUNK], bf16, tag="vs")
                    for n in range(A_CHUNK // 512):
                        ps = ps512.tile([128, 512], f32, tag="ps")
                        for cb in range(2):
                            nc.tensor.matmul(ps, wv_sb[:, cb, :],
                                             vc[:, cb, n * 512:(n + 1) * 512],
                                             start=(cb == 0), stop=(cb == 1))
                        if n == 0:
                            nc.vector.tensor_copy(out=vs[:, 0:512], in_=ps)
                        else:
                            nc.scalar.copy(out=vs[:, 512:1024], in_=ps)
                    nc.scalar.dma_start(out=v_flat[:, sl], in_=vs)

                def q_chunk(t):
                    qc = q_in_pool.tile([128, 2, QCH, C], f16, tag="qc")
                    nc.sync.dma_start(out=qc, in_=qv_in[:, :, t * QCH:(t + 1) * QCH, :])
                    for hl in range(QCH):
                        h = t * QCH + hl
                        pq = ps512.tile([128, 2, 128], f32, tag="pq")
                        for jb in range(2):
                            for cb in range(2):
                                nc.tensor.matmul(
                                    pq[:, jb, :],
                                    qc[:, cb, hl, jb * 128:(jb + 1) * 128],
                                    wq_sb[:, cb, :],
                                    start=(cb == 0), stop=(cb == 1))
                        # drain [j, (jb, i)] -> qT[:, :, h, :] (contiguous)
                        # with the uniform +bq[i] folded in (DVE tensor_add;
                        # ACT has no tensor_tensor)
                        nc.vector.tensor_add(
                            out=qT[:, h, :, :], in0=pq, in1=bqbc_sb)

                for t in range(C // QCH):
                    v_chunk(t)
                    q_chunk(t)
                # ---- Phase B: k conv + attention ----
                def kc_load(ic):
                    kc = b_kin.tile([128, 2, KCH], f16, tag="kc")
                    nc.scalar.dma_start(
                        out=kc, in_=kv_in[:, :, ic * KCH:(ic + 1) * KCH])
                    return kc

                def vt_load(ic):
                    i0 = ic * ICHUNK
                    vt = b_vt.tile([128, ICHUNK, 2, C + 8], bf16, tag="vt")
                    nc.gpsimd.dma_start(
                        out=vt[:, :, :, 0:C],
                        in_=v_scr[i0:i0 + ICHUNK].rearrange(
                            "io (jb j) w -> j io jb w", j=128))
                    nc.gpsimd.memset(vt[:, :, :, C:C + 1], 2.0)
                    return vt

                NCH = HALF // ICHUNK
                LEAD = 2

                def k_conv(kc):
                    ksb = b_ksb.tile([128, 2, ICHUNK, C], f16, tag="ksb")
                    for jb in range(2):
                        for n in range(KCH // 512):
                            ps = ps512.tile([128, 512], f32, tag="ps")
                            for cb in range(2):
                                nc.tensor.matmul(
                                    ps, wk_sb[:, cb, jb * 128:(jb + 1) * 128],
                                    kc[:, cb, n * 512:(n + 1) * 512],
                                    start=(cb == 0), stop=(cb == 1))
                            if (jb + n) % 2 == 0:
                                nc.vector.tensor_copy(
                                    out=ksb[:, jb, n * 2:(n + 1) * 2, :], in_=ps)
                            else:
                                nc.scalar.copy(
                                    out=ksb[:, jb, n * 2:(n + 1) * 2, :], in_=ps)
                    return ksb

                kc_pend = {ic: kc_load(ic) for ic in range(LEAD + 2)}
                vt_pend = {0: vt_load(0)}
                # k-conv lead: run during phase-1's tail to keep the PE
                # dense there (HAM) and thin phase B's PE load
                ksb_pend = {ic: k_conv(kc_pend.pop(ic)) for ic in range(LEAD)}
                for ic in range(NCH):
                    i0 = ic * ICHUNK
                    if ic + 2 < NCH and ic + LEAD + 2 < NCH:
                        kc_pend[ic + LEAD + 2] = kc_load(ic + LEAD + 2)
                    vt = vt_pend.pop(ic)
                    if ic + 1 < NCH:
                        vt_pend[ic + 1] = vt_load(ic + 1)
                    if ic in ksb_pend:
                        ksb = ksb_pend.pop(ic)
                    else:
                        ksb = k_conv(kc_pend.pop(ic))
                    ob = b_ob.tile([128, ICHUNK, 2, C], f16, tag="ob")
                    for io in range(ICHUNK):
                        i_loc = i0 + io
                        et = b_et.tile([128, 2, C], bf16, tag="et")
                        psc = b_psc.tile([128, 2, C], f32, tag="psc")
                        for wb in range(2):
                            for jb in range(2):
                                nc.tensor.matmul(
                                    psc[:, wb, :],
                                    ksb[:, jb, io, wb * 128:(wb + 1) * 128],
                                    qT[:, :, jb, i_loc],
                                    start=(jb == 0), stop=(jb == 1))
                        # qT carries +bq[i], so exp needs no bias and
                        # batches both w-blocks in one 512-elem pass
                        nc.scalar.activation(out=et, in_=psc, func=Exp)
                        for hb in range(2):
                            po = b_po.tile([128, C + 1], f32, tag="po")
                            for wb in range(2):
                                nc.tensor.matmul(
                                    po, et[:, wb, hb * 128:(hb + 1) * 128],
                                    vt[:, io, wb, 0:C + 1],
                                    start=(wb == 0), stop=(wb == 1))
                            rs = b_rs.tile([128, 1], f32, tag="rs")
                            nc.vector.reciprocal(out=rs, in_=po[:, C:C + 1])
                            # sigmoid(y) = 0.5*tanh(y/2)+0.5; y/2 = po*rs +
                            # bv[i]/2 (rs = 0.5/rowsum via vt's 2.0-column)
                            nc.scalar.activation(
                                out=ob[:, io, hb, :], in_=po[:, 0:C],
                                func=Tanh, scale=rs,
                                bias=bvbc_sb[:, i_loc:i_loc + 1])
                    nc.gpsimd.tensor_scalar(
                        out=ob, in0=ob,
                        scalar1=0.5, scalar2=0.5,
                        op0=mybir.AluOpType.mult,
                        op1=mybir.AluOpType.add)
                    nc.sync.dma_start(
                        out=out_b[i0:i0 + ICHUNK].rearrange(
                            "io (hb h) w -> h io hb w", h=128),
                        in_=ob)

            if repeat == 1:
                body()
            else:
                with tc.For_i(0, repeat, 1) as it:
                    body(it)

    nc.compile()
    _CACHE[key] = nc
    return nc


def make_in_maps(inputs):
    query = np.asarray(inputs["query"], dtype=np.float32)
    key_in = np.asarray(inputs["key_in"], dtype=np.float32)
    value = np.asarray(inputs["value"], dtype=np.float32)
    Wq = np.asarray(inputs["Wq"], dtype=np.float32)
    Wk = np.asarray(inputs["Wk"], dtype=np.float32)
    Wv = np.asarray(inputs["Wv"], dtype=np.float32)
    bq = np.asarray(inputs["bq"], dtype=np.float32)
    bv = np.asarray(inputs["bv"], dtype=np.float32)
    in_maps = []
    for core in range(N_CORES):
        b, g = core // 2, core % 2
        sl = slice(g * HALF, (g + 1) * HALF)
        in_maps.append({
            "query_b": np.ascontiguousarray(query[b], dtype=np.float16),
            "key_h": np.ascontiguousarray(key_in[b][:, sl, :], dtype=np.float16),
            "value_b": np.ascontiguousarray(value[b], dtype=np.float16),
            "wqT": np.ascontiguousarray(Wq[sl, :].T, dtype=np.float16),
            "wkT": np.ascontiguousarray(Wk.T, dtype=np.float16),
            "wvT": np.ascontiguousarray(Wv[sl, :].T, dtype=np.float16),
            "bq_bc": np.ascontiguousarray(
                np.broadcast_to(bq[sl].astype(np.float32), (128, 2, HALF))),
            "bvh_bc": np.ascontiguousarray(
                np.broadcast_to((bv[sl] / 2).astype(np.float32), (128, HALF))),
        })
    return in_maps


def kernel(query, key_in, value, Wq, bq, Wk, bk, Wv, bv):
    nc = build_nc()
    in_maps = make_in_maps(dict(query=query, key_in=key_in, value=value,
                                Wq=Wq, bq=bq, Wk=Wk, bk=bk, Wv=Wv, bv=bv))
    res = run_bass_kernel_spmd(nc, in_maps, core_ids=list(range(N_CORES)))
    out = np.empty((4, C, C, C), dtype=np.float32)
    for core in range(N_CORES):
        b, g = core // 2, core % 2
        out[b, g * HALF:(g + 1) * HALF] = res.results[core]["out_b"].astype(np.float32)
    return out


# revision 14
# speedup vs baseline: 1.0753x; 1.0753x over previous
"""Trainium2 Bass kernel for nn_AttentionBlock (B=4, C=H=W=S=256). v5

reference:
  q = Wq @ query + bq   (1x1 conv over channel dim)
  k = Wk @ key_in + bk
  v = Wv @ value + bv
  scores[b,i,h,w] = sum_j q[b,i,h,j] * k[b,j,i,w]
  attn = softmax(scores, -1)
  out[b,i,h,w] = sum_j attn[b,i,h,j] * v[b,i,j,w]
  return sigmoid(out)

Sharding: 8 cores = (b, g) with b=core//2, g=core%2; each core computes
out[b, g*128:(g+1)*128, :, :].

v5 design (vs v4 baseline, 618us measured on HW):
  - q conv computed DIRECTLY TRANSPOSED: per (h, jb) the query tile
    [c, j] is the stationary operand and WqT [c, i] the moving one, so
    the PE emits qT[j, i] straight into PSUM. Kills the 512 identity
    transposes (~27us PE) and the 176us of DVE COPY + scalar copyback
    drains that v4 spent rotating the conv output.
  - bias algebra (validated exact in fp64):
      * bk drops entirely: its score contribution is constant along the
        softmax axis w, and softmax is shift-invariant.
      * bq folds into the scores matmul: qT's 257th h-column holds
        bq[i]; the matmul then emits bq[i]*colsum_k0[i,w] in psc col
        256, which is exactly the per-partition (w) bias exp needs.
      * bv folds into the tanh bias: sum_j attn = 1, so attn@(v0+bv) =
        attn@v0 + bv[i]; tanh arg gets + bv[i]/2 via a broadcast tile.
      * softmax normalization + sigmoid run as ONE activation per
        (io,hb): ob = tanh(po * rs + bv[i]/2) with scale=rs=0.5/rowsum
        (rowsum from vt's 2.0-column), then a Pool-engine *0.5+0.5
        affine per chunk. No yt staging tile, no batched-tanh pass.
  - phase order V -> Q -> B so the v readback DMA (chunked, i-major)
    can stream during phase Q; k input chunks prefetch 3 deep, v
    readbacks 2 deep (just-in-time: V and Q windows are DMA-bound, so
    deeper prefetch only steals their bandwidth).
  - drains split DVE/ACT; DMA issue spread over SP (loads), ACT
    (stores), DVE (v readback), Pool (weights) queues.
  => per-core HBM traffic unchanged (~134MB: q/k/v inputs f16 84MB,
    v round-trip bf16 33.5MB, out f16 16.8MB) but phases V/Q run as
    pure DMA races while phase B is ACT-bound with its DMA hidden.
"""

import numpy as np

import concourse.bass as bass
import concourse.tile as tile
from concourse import bacc, mybir
from concourse.bass_utils import run_bass_kernel_spmd

C = 256
HALF = 128          # output channels per core
N_CORES = 8
ICHUNK = 4          # i values per phase-B chunk
A_CHUNK = 1024      # flattened spatial elems per phase-V chunk
QCH = 4             # h rows per phase-Q chunk

_CACHE = {}


def build_nc(repeat=1):
    key = ("nc", repeat)
    if key in _CACHE:
        return _CACHE[key]
    f32 = mybir.dt.float32
    f16 = mybir.dt.float16
    bf16 = mybir.dt.bfloat16

    nc = bacc.Bacc("TRN2", target_bir_lowering=False, debug=False,
                   num_devices=N_CORES)

    query_b = nc.dram_tensor("query_b", [C, C, C], f16, kind="ExternalInput").ap()
    key_h = nc.dram_tensor("key_h", [C, HALF, C], f16, kind="ExternalInput").ap()
    value_b = nc.dram_tensor("value_b", [C, C, C], f16, kind="ExternalInput").ap()
    wqT = nc.dram_tensor("wqT", [C, HALF], f16, kind="ExternalInput").ap()
    wkT = nc.dram_tensor("wkT", [C, C], f16, kind="ExternalInput").ap()
    wvT = nc.dram_tensor("wvT", [C, HALF], f16, kind="ExternalInput").ap()
    bq_bc = nc.dram_tensor("bq_bc", [128, 2, HALF], f32, kind="ExternalInput").ap()
    bvh_bc = nc.dram_tensor("bvh_bc", [128, HALF], f32, kind="ExternalInput").ap()
    out_b = nc.dram_tensor("out_b", [HALF, C, C], f16, kind="ExternalOutput").ap()

    v_scr = nc.dram_tensor("v_scr", [HALF, C, C], bf16).ap()

    HJ = C * C  # 65536
    KCH = ICHUNK * C  # flattened (il, w) per phase-B chunk

    qv_in = query_b.rearrange("(cb c) h j -> c cb h j", c=128)
    vv_in = value_b.rearrange("(cb c) j w -> c cb (j w)", c=128)
    kv_in = key_h.rearrange("(cb c) il w -> c cb (il w)", c=128)
    v_flat = v_scr.rearrange("i j w -> i (j w)")

    Exp = mybir.ActivationFunctionType.Exp
    Tanh = mybir.ActivationFunctionType.Tanh

    with tile.TileContext(nc) as tc:
        with (
            tc.tile_pool(name="weights", bufs=1) as wpool,
            tc.tile_pool(name="v_in", bufs=3) as v_in_pool,
            tc.tile_pool(name="v_out", bufs=2) as v_out_pool,
            tc.tile_pool(name="q_in", bufs=2) as q_in_pool,
            tc.tile_pool(name="ps512", bufs=2, space="PSUM") as ps512,
            tc.tile_pool(name="b_kin", bufs=3) as b_kin,
            tc.tile_pool(name="b_ksb", bufs=4) as b_ksb,
            tc.tile_pool(name="b_vt", bufs=3) as b_vt,
            tc.tile_pool(name="b_et", bufs=2) as b_et,
            tc.tile_pool(name="b_ob", bufs=2) as b_ob,
            tc.tile_pool(name="b_rs", bufs=8) as b_rs,
            tc.tile_pool(name="b_psc", bufs=2, space="PSUM") as b_psc,
            tc.tile_pool(name="b_po", bufs=2, space="PSUM") as b_po,
        ):
            wq_sb = wpool.tile([128, 2, HALF], f16)
            wk_sb = wpool.tile([128, 2, C], f16)
            wv_sb = wpool.tile([128, 2, HALF], f16)
            nc.gpsimd.dma_start(out=wq_sb, in_=wqT.rearrange("(cb c) i -> c cb i", c=128))
            nc.gpsimd.dma_start(out=wk_sb, in_=wkT.rearrange("(cb c) j -> c cb j", c=128))
            nc.gpsimd.dma_start(out=wv_sb, in_=wvT.rearrange("(cb c) i -> c cb i", c=128))
            bqbc_sb = wpool.tile([128, 2, HALF], f32)
            bvbc_sb = wpool.tile([128, HALF], f32)
            nc.gpsimd.dma_start(out=bqbc_sb, in_=bq_bc)
            nc.gpsimd.dma_start(out=bvbc_sb, in_=bvh_bc)
            # SBUF-resident transposed q for the whole kernel, i-last so
            # the conv drains write contiguous 128-elem runs:
            # qT[j, h, jb, i] = q0[i, h, jb*128+j] + bq[i]  (uniform +bq[i]
            # IS the score bias: sum_j k0*(q0+bq) = scores0 + bq*colsum_k0)
            qT = wpool.tile([128, C, 2, HALF], f16)

            def body(_it=None):
                # ---- Phase 1: V and Q convs interleaved per chunk ----
                # V: natural [i,(j,w)] conv -> DRAM round-trip (ap-512,
                # weights stationary). Q: direct-transposed conv (query
                # tile stationary, ap-128) -> resident qT. Interleaving
                # keeps the PE denser (HAM throttle) and both input
                # streams + the v_scr writeback on the DMA rings at once.
                # V runs 2 chunks per t for t<32 so v_scr completes at the
                # halfway point and phase-B v readbacks stream during the
                # entire second half of phase 1.
                def v_chunk(t):
                    sl = slice(t * A_CHUNK, (t + 1) * A_CHUNK)
                    vc = v_in_pool.tile([128, 2, A_CHUNK], f16, tag="vc")
                    nc.sync.dma_start(out=vc, in_=vv_in[:, :, sl])
                    vs = v_out_pool.tile([128, A_CHUNK], bf16, tag="vs")
                    for n in range(A_CHUNK // 512):
                        ps = ps512.tile([128, 512], f32, tag="ps")
                        for cb in range(2):
                            nc.tensor.matmul(ps, wv_sb[:, cb, :],
                                             vc[:, cb, n * 512:(n + 1) * 512],
                                             start=(cb == 0), stop=(cb == 1))
                        if n == 0:
                            nc.vector.tensor_copy(out=vs[:, 0:512], in_=ps)
                        else:
                            nc.scalar.copy(out=vs[:, 512:1024], in_=ps)
                    nc.scalar.dma_start(out=v_flat[:, sl], in_=vs)

                def q_chunk(t):
                    qc = q_in_pool.tile([128, 2, QCH, C], f16, tag="qc")
                    nc.sync.dma_start(out=qc, in_=qv_in[:, :, t * QCH:(t + 1) * QCH, :])
                    for hl in range(QCH):
                        h = t * QCH + hl
                        pq = ps512.tile([128, 2, 128], f32, tag="pq")
                        for jb in range(2):
                            for cb in range(2):
                                nc.tensor.matmul(
                                    pq[:, jb, :],
                                    qc[:, cb, hl, jb * 128:(jb + 1) * 128],
                                    wq_sb[:, cb, :],
                                    start=(cb == 0), stop=(cb == 1))
                        # drain [j, (jb, i)] -> qT[:, :, h, :] (contiguous)
                        # with the uniform +bq[i] folded in (DVE tensor_add;
                        # ACT has no tensor_tensor)
                        nc.vector.tensor_add(
                            out=qT[:, h, :, :], in0=pq, in1=bqbc_sb)

                for t in range(C // QCH):
                    v_chunk(t)
                    q_chunk(t)
                # ---- Phase B: k conv + attention ----
                def kc_load(ic):
                    kc = b_kin.tile([128, 2, KCH], f16, tag="kc")
                    nc.scalar.dma_start(
                        out=kc, in_=kv_in[:, :, ic * KCH:(ic + 1) * KCH])
                    return kc

                def vt_load(ic):
                    i0 = ic * ICHUNK
                    vt = b_vt.tile([128, ICHUNK, 2, C + 8], bf16, tag="vt")
                    nc.gpsimd.dma_start(
                        out=vt[:, :, :, 0:C],
                        in_=v_scr[i0:i0 + ICHUNK].rearrange(
                            "io (jb j) w -> j io jb w", j=128))
                    nc.gpsimd.memset(vt[:, :, :, C:C + 1], 2.0)
                    return vt

                NCH = HALF // ICHUNK
                LEAD = 0

                def k_conv(kc):
                    ksb = b_ksb.tile([128, 2, ICHUNK, C], f16, tag="ksb")
                    for jb in range(2):
                        for n in range(KCH // 512):
                            ps = ps512.tile([128, 512], f32, tag="ps")
                            for cb in range(2):
                                nc.tensor.matmul(
                                    ps, wk_sb[:, cb, jb * 128:(jb + 1) * 128],
                                    kc[:, cb, n * 512:(n + 1) * 512],
                                    start=(cb == 0), stop=(cb == 1))
                            if (jb + n) % 2 == 0:
                                nc.vector.tensor_copy(
                                    out=ksb[:, jb, n * 2:(n + 1) * 2, :], in_=ps)
                            else:
                                nc.scalar.copy(
                                    out=ksb[:, jb, n * 2:(n + 1) * 2, :], in_=ps)
                    return ksb

                kc_pend = {ic: kc_load(ic) for ic in range(LEAD + 2)}
                vt_pend = {0: vt_load(0), 1: vt_load(1)}
                # k-conv lead: run during phase-1's tail to keep the PE
                # dense there (HAM) and thin phase B's PE load
                ksb_pend = {ic: k_conv(kc_pend.pop(ic)) for ic in range(LEAD)}
                for ic in range(NCH):
                    i0 = ic * ICHUNK
                    if ic + 2 < NCH and ic + LEAD + 2 < NCH:
                        kc_pend[ic + LEAD + 2] = kc_load(ic + LEAD + 2)
                    vt = vt_pend.pop(ic)
                    if ic + 2 < NCH:
                        vt_pend[ic + 2] = vt_load(ic + 2)
                    if ic in ksb_pend:
                        ksb = ksb_pend.pop(ic)
                    else:
                        ksb = k_conv(kc_pend.pop(ic))
                    ob = b_ob.tile([128, ICHUNK, 2, C], f16, tag="ob")
                    for io in range(ICHUNK):
                        i_loc = i0 + io
                        et = b_et.tile([128, 2, C], bf16, tag="et")
                        psc = b_psc.tile([128, 2, C], f32, tag="psc")
                        for wb in range(2):
                            for jb in range(2):
                                nc.tensor.matmul(
                                    psc[:, wb, :],
                                    ksb[:, jb, io, wb * 128:(wb + 1) * 128],
                                    qT[:, :, jb, i_loc],
                                    start=(jb == 0), stop=(jb == 1))
                        # qT carries +bq[i], so exp needs no bias and
                        # batches both w-blocks in one 512-elem pass
                        nc.scalar.activation(out=et, in_=psc, func=Exp)
                        for hb in range(2):
                            po = b_po.tile([128, C + 1], f32, tag="po")
                            for wb in range(2):
                                nc.tensor.matmul(
                                    po, et[:, wb, hb * 128:(hb + 1) * 128],
                                    vt[:, io, wb, 0:C + 1],
                                    start=(wb == 0), stop=(wb == 1))
                            rs = b_rs.tile([128, 1], f32, tag="rs")
                            nc.vector.reciprocal(out=rs, in_=po[:, C:C + 1])
                            # sigmoid(y) = 0.5*tanh(y/2)+0.5; y/2 = po*rs +
                            # bv[i]/2 (rs = 0.5/rowsum via vt's 2.0-column)
                            nc.scalar.activation(
                                out=ob[:, io, hb, :], in_=po[:, 0:C],
                                func=Tanh, scale=rs,
                                bias=bvbc_sb[:, i_loc:i_loc + 1])
                    nc.gpsimd.tensor_scalar(
                        out=ob, in0=ob,
                        scalar1=0.5, scalar2=0.5,
                        op0=mybir.AluOpType.mult,
                        op1=mybir.AluOpType.add)
                    nc.sync.dma_start(
                        out=out_b[i0:i0 + ICHUNK].rearrange(
                            "io (hb h) w -> h io hb w", h=128),
                        in_=ob)

            if repeat == 1:
                body()
            else:
                with tc.For_i(0, repeat, 1) as it:
                    body(it)

    nc.compile()
    _CACHE[key] = nc
    return nc


def make_in_maps(inputs):
    query = np.asarray(inputs["query"], dtype=np.float32)
    key_in = np.asarray(inputs["key_in"], dtype=np.float32)
    value = np.asarray(inputs["value"], dtype=np.float32)
    Wq = np.asarray(inputs["Wq"], dtype=np.float32)
    Wk = np.asarray(inputs["Wk"], dtype=np.float32)
    Wv = np.asarray(inputs["Wv"], dtype=np.float32)
    bq = np.asarray(inputs["bq"], dtype=np.float32)
    bv = np.asarray(inputs["bv"], dtype=np.float32)
    in_maps = []
    for core in range(N_CORES):
        b, g = core // 2, core % 2
        sl = slice(g * HALF, (g + 1) * HALF)
        in_maps.append({
            "query_b": np.ascontiguousarray(query[b], dtype=np.float16),
            "key_h": np.ascontiguousarray(key_in[b][:, sl, :], dtype=np.float16),
            "value_b": np.ascontiguousarray(value[b], dtype=np.float16),
            "wqT": np.ascontiguousarray(Wq[sl, :].T, dtype=np.float16),
            "wkT": np.ascontiguousarray(Wk.T, dtype=np.float16),
            "wvT": np.ascontiguousarray(Wv[sl, :].T, dtype=np.float16),
            "bq_bc": np.ascontiguousarray(
                np.broadcast_to(bq[sl].astype(np.float32), (128, 2, HALF))),
            "bvh_bc": np.ascontiguousarray(
                np.broadcast_to((bv[sl] / 2).astype(np.float32), (128, HALF))),
        })
    return in_maps


def kernel(query, key_in, value, Wq, bq, Wk, bk, Wv, bv):
    nc = build_nc()
    in_maps = make_in_maps(dict(query=query, key_in=key_in, value=value,
                                Wq=Wq, bq=bq, Wk=Wk, bk=bk, Wv=Wv, bv=bv))
    res = run_bass_kernel_spmd(nc, in_maps, core_ids=list(range(N_CORES)))
    out = np.empty((4, C, C, C), dtype=np.float32)
    for core in range(N_CORES):
        b, g = core // 2, core % 2
        out[b, g * HALF:(g + 1) * HALF] = res.results[core]["out_b"].astype(np.float32)
    return out


# revision 15
# speedup vs baseline: 1.0794x; 1.0038x over previous
"""Trainium2 Bass kernel for nn_AttentionBlock (B=4, C=H=W=S=256). v10

reference:
  q = Wq @ query + bq   (1x1 conv over channel dim)
  k = Wk @ key_in + bk
  v = Wv @ value + bv
  scores[b,i,h,w] = sum_j q[b,i,h,j] * k[b,j,i,w]
  attn = softmax(scores, -1)
  out[b,i,h,w] = sum_j attn[b,i,h,j] * v[b,i,j,w]
  return sigmoid(out)

Sharding: 8 cores = (b, g) with b=core//2, g=core%2; each core computes
out[b, g*128:(g+1)*128, :, :].

Design (measured-driven; HW baselines: v4 618us, v5 776, v7 746, v9 737):
  - Phase 1 interleaves the v conv (natural layout, ap-512, DRAM
    round-trip) and the DIRECT-TRANSPOSED q conv (query tile stationary,
    ap-128) uniformly: 1 v-chunk + 1 q-chunk per iteration keeps both
    input streams + the v_scr writeback saturating the DMA rings
    (~300GB/s measured) and the PE dense enough for the HAM clock gate.
  - qT is SBUF-resident [j, jb, i, h] with h LAST: the phase-B scores
    matmuls then stream qT contiguously (a strided moving operand
    measures 2.7x slower: 431 vs 162 ns/iter warm, independent of
    stride size). The transpose tax moves to phase 1's PSUM->SBUF
    drains (strided h-column writes, ~1.1us each) where DVE/ACT have
    headroom under the DMA bound -- placing the tax on phase B's PE
    (v7/v9, i-last qT) measured ~120us worse.
  - bias algebra (validated exact in fp64):
      * bk drops entirely: its score contribution is constant along the
        softmax axis w, and softmax is shift-invariant.
      * bq folds into the scores matmul: qT's 257th h-column holds
        bq[i]; the matmul then emits bq[i]*colsum_k0[i,w] in psc col
        256, which is the per-partition (w) bias exp needs (bounced to
        SBUF via a 1-col DVE copy; ACT bias APs must live in SBUF).
      * bv folds into the tanh bias: sum_j attn = 1, so attn@(v0+bv) =
        attn@v0 + bv[i]; tanh arg gets + bv[i]/2 via a broadcast tile.
      * softmax normalization + sigmoid run as ONE activation per
        (io,hb): ob = tanh(po * rs + bv[i]/2) with scale=rs=0.5/rowsum
        (rowsum via vt's 2.0-column), then a Pool *0.5+0.5 affine per
        chunk. No separate normalize pass, no batched-tanh staging.
  - phase-B streams (k chunks, v readbacks) prefetch 2-3 deep; the k
    conv output ring is 4 deep so the scores/attnv pipeline doesn't
    stall at chunk boundaries. DMA issue spread over SP (loads + out
    stores), ACT (v_scr stores, kc), Pool/SWDGE (v readbacks, weights).
  => per-core HBM traffic ~134MB (inputs f16 84MB, v round-trip bf16
    33.5MB, out f16 16.8MB); phase 1 runs at the DMA roofline, phase B
    is PE/ACT-bound with contiguous matmul streams.
"""

import numpy as np

import concourse.bass as bass
import concourse.tile as tile
from concourse import bacc, mybir
from concourse.bass_utils import run_bass_kernel_spmd

C = 256
HALF = 128          # output channels per core
N_CORES = 8
ICHUNK = 4          # i values per phase-B chunk
A_CHUNK = 1024      # flattened spatial elems per phase-1 v chunk
QCH = 4             # h rows per phase-1 q chunk

_CACHE = {}


def build_nc(repeat=1):
    key = ("nc", repeat)
    if key in _CACHE:
        return _CACHE[key]
    f32 = mybir.dt.float32
    f16 = mybir.dt.float16
    bf16 = mybir.dt.bfloat16

    nc = bacc.Bacc("TRN2", target_bir_lowering=False, debug=False,
                   num_devices=N_CORES)

    query_b = nc.dram_tensor("query_b", [C, C, C], f16, kind="ExternalInput").ap()
    key_h = nc.dram_tensor("key_h", [C, HALF, C], f16, kind="ExternalInput").ap()
    value_b = nc.dram_tensor("value_b", [C, C, C], f16, kind="ExternalInput").ap()
    wqT = nc.dram_tensor("wqT", [C, HALF], f16, kind="ExternalInput").ap()
    wkT = nc.dram_tensor("wkT", [C, C], f16, kind="ExternalInput").ap()
    wvT = nc.dram_tensor("wvT", [C, HALF], f16, kind="ExternalInput").ap()
    bq_bc = nc.dram_tensor("bq_bc", [128, HALF], f16, kind="ExternalInput").ap()
    bvh_bc = nc.dram_tensor("bvh_bc", [128, HALF], f32, kind="ExternalInput").ap()
    out_b = nc.dram_tensor("out_b", [HALF, C, C], f16, kind="ExternalOutput").ap()

    v_scr = nc.dram_tensor("v_scr", [HALF, C, C], bf16).ap()

    HJ = C * C  # 65536
    KCH = ICHUNK * C  # flattened (il, w) per phase-B chunk

    qv_in = query_b.rearrange("(cb c) h j -> c cb h j", c=128)
    vv_in = value_b.rearrange("(cb c) j w -> c cb (j w)", c=128)
    kv_in = key_h.rearrange("(cb c) il w -> c cb (il w)", c=128)
    v_flat = v_scr.rearrange("i j w -> i (j w)")

    Exp = mybir.ActivationFunctionType.Exp
    Tanh = mybir.ActivationFunctionType.Tanh

    with tile.TileContext(nc) as tc:
        with (
            tc.tile_pool(name="weights", bufs=1) as wpool,
            tc.tile_pool(name="v_in", bufs=3) as v_in_pool,
            tc.tile_pool(name="v_out", bufs=2) as v_out_pool,
            tc.tile_pool(name="q_in", bufs=2) as q_in_pool,
            tc.tile_pool(name="ps512", bufs=2, space="PSUM") as ps512,
            tc.tile_pool(name="b_kin", bufs=3) as b_kin,
            tc.tile_pool(name="b_ksb", bufs=4) as b_ksb,
            tc.tile_pool(name="b_vt", bufs=3) as b_vt,
            tc.tile_pool(name="b_et", bufs=2) as b_et,
            tc.tile_pool(name="b_ob", bufs=2) as b_ob,
            tc.tile_pool(name="b_rs", bufs=8) as b_rs,
            tc.tile_pool(name="b_eb", bufs=8) as b_eb,
            tc.tile_pool(name="b_psc", bufs=2, space="PSUM") as b_psc,
            tc.tile_pool(name="b_po", bufs=2, space="PSUM") as b_po,
        ):
            wq_sb = wpool.tile([128, 2, HALF], f16)
            wk_sb = wpool.tile([128, 2, C], f16)
            wv_sb = wpool.tile([128, 2, HALF], f16)
            nc.gpsimd.dma_start(out=wq_sb, in_=wqT.rearrange("(cb c) i -> c cb i", c=128))
            nc.gpsimd.dma_start(out=wk_sb, in_=wkT.rearrange("(cb c) j -> c cb j", c=128))
            nc.gpsimd.dma_start(out=wv_sb, in_=wvT.rearrange("(cb c) i -> c cb i", c=128))
            bqbc_sb = wpool.tile([128, HALF], f16)
            bvbc_sb = wpool.tile([128, HALF], f32)
            nc.gpsimd.dma_start(out=bqbc_sb, in_=bq_bc)
            nc.gpsimd.dma_start(out=bvbc_sb, in_=bvh_bc)
            # SBUF-resident transposed q, h last (contiguous scores rhs):
            # qT[j, jb, i, h] = q0[i, h, jb*128+j]; column h=256 = bq[i]
            qT = wpool.tile([128, 2, HALF, C + 1], f16)

            def body(_it=None):
                # ---- Phase 1: v conv + direct-transposed q conv ----
                def v_chunk(t):
                    sl = slice(t * A_CHUNK, (t + 1) * A_CHUNK)
                    vc = v_in_pool.tile([128, 2, A_CHUNK], f16, tag="vc")
                    nc.sync.dma_start(out=vc, in_=vv_in[:, :, sl])
                    vs = v_out_pool.tile([128, A_CHUNK], bf16, tag="vs")
                    for n in range(A_CHUNK // 512):
                        ps = ps512.tile([128, 512], f32, tag="ps")
                        for cb in range(2):
                            nc.tensor.matmul(ps, wv_sb[:, cb, :],
                                             vc[:, cb, n * 512:(n + 1) * 512],
                                             start=(cb == 0), stop=(cb == 1))
                        if n == 0:
                            nc.vector.tensor_copy(out=vs[:, 0:512], in_=ps)
                        else:
                            nc.scalar.copy(out=vs[:, 512:1024], in_=ps)
                    nc.scalar.dma_start(out=v_flat[:, sl], in_=vs)

                def q_chunk(t):
                    qc = q_in_pool.tile([128, 2, QCH, C], f16, tag="qc")
                    nc.sync.dma_start(out=qc, in_=qv_in[:, :, t * QCH:(t + 1) * QCH, :])
                    for hl in range(QCH):
                        h = t * QCH + hl
                        pq = ps512.tile([128, 2, 128], f32, tag="pq")
                        for jb in range(2):
                            for cb in range(2):
                                nc.tensor.matmul(
                                    pq[:, jb, :],
                                    qc[:, cb, hl, jb * 128:(jb + 1) * 128],
                                    wq_sb[:, cb, :],
                                    start=(cb == 0), stop=(cb == 1))
                        # strided h-column drain (the transpose tax lives
                        # here, under phase 1's DMA bound), alternating
                        # engines so consecutive h's drain concurrently
                        if hl % 2 == 0:
                            nc.vector.tensor_copy(out=qT[:, :, :, h], in_=pq)
                        else:
                            nc.scalar.copy(out=qT[:, :, :, h], in_=pq)

                for t in range(C // QCH):
                    v_chunk(t)
                    q_chunk(t)
                # bias column: qT[:, jb, i, 256] = bq[i]
                nc.vector.tensor_copy(out=qT[:, 0, :, C], in_=bqbc_sb)
                nc.vector.tensor_copy(out=qT[:, 1, :, C], in_=bqbc_sb)

                # ---- Phase B: k conv + attention ----
                def kc_load(ic):
                    kc = b_kin.tile([128, 2, KCH], f16, tag="kc")
                    nc.scalar.dma_start(
                        out=kc, in_=kv_in[:, :, ic * KCH:(ic + 1) * KCH])
                    return kc

                def vt_load(ic):
                    i0 = ic * ICHUNK
                    vt = b_vt.tile([128, ICHUNK, 2, C + 8], bf16, tag="vt")
                    nc.gpsimd.dma_start(
                        out=vt[:, :, :, 0:C],
                        in_=v_scr[i0:i0 + ICHUNK].rearrange(
                            "io (jb j) w -> j io jb w", j=128))
                    nc.gpsimd.memset(vt[:, :, :, C:C + 1], 2.0)
                    return vt

                def k_conv(kc):
                    ksb = b_ksb.tile([128, 2, ICHUNK, C], f16, tag="ksb")
                    for jb in range(2):
                        for n in range(KCH // 512):
                            ps = ps512.tile([128, 512], f32, tag="ps")
                            for cb in range(2):
                                nc.tensor.matmul(
                                    ps, wk_sb[:, cb, jb * 128:(jb + 1) * 128],
                                    kc[:, cb, n * 512:(n + 1) * 512],
                                    start=(cb == 0), stop=(cb == 1))
                            if (jb + n) % 2 == 0:
                                nc.vector.tensor_copy(
                                    out=ksb[:, jb, n * 2:(n + 1) * 2, :], in_=ps)
                            else:
                                nc.scalar.copy(
                                    out=ksb[:, jb, n * 2:(n + 1) * 2, :], in_=ps)
                    return ksb

                NCH = HALF // ICHUNK
                kc_pend = {0: kc_load(0), 1: kc_load(1)}
                vt_pend = {0: vt_load(0), 1: vt_load(1)}
                for ic in range(NCH):
                    i0 = ic * ICHUNK
                    if ic + 2 < NCH:
                        kc_pend[ic + 2] = kc_load(ic + 2)
                    vt = vt_pend.pop(ic)
                    if ic + 2 < NCH:
                        vt_pend[ic + 2] = vt_load(ic + 2)
                    ksb = k_conv(kc_pend.pop(ic))
                    ob = b_ob.tile([128, ICHUNK, 2, C], f16, tag="ob")
                    for io in range(ICHUNK):
                        i_loc = i0 + io
                        et = b_et.tile([128, 2, C], bf16, tag="et")
                        for wb in range(2):
                            psc = b_psc.tile([128, C + 1], f32, tag="psc")
                            for jb in range(2):
                                nc.tensor.matmul(
                                    psc,
                                    ksb[:, jb, io, wb * 128:(wb + 1) * 128],
                                    qT[:, jb, i_loc, :],
                                    start=(jb == 0), stop=(jb == 1))
                            # col 256 = bq[i]*colsum_k0[i, w-block]: the
                            # per-partition exp bias, bounced to SBUF
                            eb = b_eb.tile([128, 1], f32, tag="eb")
                            nc.vector.tensor_copy(out=eb, in_=psc[:, C:C + 1])
                            nc.scalar.activation(
                                out=et[:, wb, :], in_=psc[:, 0:C],
                                func=Exp, bias=eb)
                        for hb in range(2):
                            po = b_po.tile([128, C + 1], f32, tag="po")
                            for wb in range(2):
                                nc.tensor.matmul(
                                    po, et[:, wb, hb * 128:(hb + 1) * 128],
                                    vt[:, io, wb, 0:C + 1],
                                    start=(wb == 0), stop=(wb == 1))
                            rs = b_rs.tile([128, 1], f32, tag="rs")
                            nc.vector.reciprocal(out=rs, in_=po[:, C:C + 1])
                            # sigmoid(y) = 0.5*tanh(y/2)+0.5; y/2 = po*rs +
                            # bv[i]/2 (rs = 0.5/rowsum via vt's 2.0-column)
                            nc.scalar.activation(
                                out=ob[:, io, hb, :], in_=po[:, 0:C],
                                func=Tanh, scale=rs,
                                bias=bvbc_sb[:, i_loc:i_loc + 1])
                    nc.gpsimd.tensor_scalar(
                        out=ob, in0=ob,
                        scalar1=0.5, scalar2=0.5,
                        op0=mybir.AluOpType.mult,
                        op1=mybir.AluOpType.add)
                    nc.sync.dma_start(
                        out=out_b[i0:i0 + ICHUNK].rearrange(
                            "io (hb h) w -> h io hb w", h=128),
                        in_=ob)

            if repeat == 1:
                body()
            else:
                with tc.For_i(0, repeat, 1) as it:
                    body(it)

    nc.compile()
    _CACHE[key] = nc
    return nc


def make_in_maps(inputs):
    query = np.asarray(inputs["query"], dtype=np.float32)
    key_in = np.asarray(inputs["key_in"], dtype=np.float32)
    value = np.asarray(inputs["value"], dtype=np.float32)
    Wq = np.asarray(inputs["Wq"], dtype=np.float32)
    Wk = np.asarray(inputs["Wk"], dtype=np.float32)
    Wv = np.asarray(inputs["Wv"], dtype=np.float32)
    bq = np.asarray(inputs["bq"], dtype=np.float32)
    bv = np.asarray(inputs["bv"], dtype=np.float32)
    in_maps = []
    for core in range(N_CORES):
        b, g = core // 2, core % 2
        sl = slice(g * HALF, (g + 1) * HALF)
        in_maps.append({
            "query_b": np.ascontiguousarray(query[b], dtype=np.float16),
            "key_h": np.ascontiguousarray(key_in[b][:, sl, :], dtype=np.float16),
            "value_b": np.ascontiguousarray(value[b], dtype=np.float16),
            "wqT": np.ascontiguousarray(Wq[sl, :].T, dtype=np.float16),
            "wkT": np.ascontiguousarray(Wk.T, dtype=np.float16),
            "wvT": np.ascontiguousarray(Wv[sl, :].T, dtype=np.float16),
            "bq_bc": np.ascontiguousarray(
                np.broadcast_to(bq[sl].astype(np.float16), (128, HALF))),
            "bvh_bc": np.ascontiguousarray(
                np.broadcast_to((bv[sl] / 2).astype(np.float32), (128, HALF))),
        })
    return in_maps


def kernel(query, key_in, value, Wq, bq, Wk, bk, Wv, bv):
    nc = build_nc()
    in_maps = make_in_maps(dict(query=query, key_in=key_in, value=value,
                                Wq=Wq, bq=bq, Wk=Wk, bk=bk, Wv=Wv, bv=bv))
    res = run_bass_kernel_spmd(nc, in_maps, core_ids=list(range(N_CORES)))
    out = np.empty((4, C, C, C), dtype=np.float32)
    for core in range(N_CORES):
        b, g = core // 2, core % 2
        out[b, g * HALF:(g + 1) * HALF] = res.results[core]["out_b"].astype(np.float32)
    return out


# revision 16
# speedup vs baseline: 1.2061x; 1.1173x over previous
"""Trainium2 Bass kernel for nn_AttentionBlock (B=4, C=H=W=S=256). v10

reference:
  q = Wq @ query + bq   (1x1 conv over channel dim)
  k = Wk @ key_in + bk
  v = Wv @ value + bv
  scores[b,i,h,w] = sum_j q[b,i,h,j] * k[b,j,i,w]
  attn = softmax(scores, -1)
  out[b,i,h,w] = sum_j attn[b,i,h,j] * v[b,i,j,w]
  return sigmoid(out)

Sharding: 8 cores = (b, g) with b=core//2, g=core%2; each core computes
out[b, g*128:(g+1)*128, :, :].

Design (measured-driven; HW baselines: v4 618us, v5 776, v7 746, v9 737):
  - Phase 1 interleaves the v conv (natural layout, ap-512, DRAM
    round-trip) and the DIRECT-TRANSPOSED q conv (query tile stationary,
    ap-128) uniformly: 1 v-chunk + 1 q-chunk per iteration keeps both
    input streams + the v_scr writeback saturating the DMA rings
    (~300GB/s measured) and the PE dense enough for the HAM clock gate.
  - qT is SBUF-resident [j, jb, i, h] with h LAST: the phase-B scores
    matmuls then stream qT contiguously (a strided moving operand
    measures 2.7x slower: 431 vs 162 ns/iter warm, independent of
    stride size). The transpose tax moves to phase 1's PSUM->SBUF
    drains (strided h-column writes, ~1.1us each) where DVE/ACT have
    headroom under the DMA bound -- placing the tax on phase B's PE
    (v7/v9, i-last qT) measured ~120us worse.
  - bias algebra (validated exact in fp64):
      * bk drops entirely: its score contribution is constant along the
        softmax axis w, and softmax is shift-invariant.
      * bq folds into the scores matmul: qT's 257th h-column holds
        bq[i]; the matmul then emits bq[i]*colsum_k0[i,w] in psc col
        256, which is the per-partition (w) bias exp needs (bounced to
        SBUF via a 1-col DVE copy; ACT bias APs must live in SBUF).
      * bv folds into the tanh bias: sum_j attn = 1, so attn@(v0+bv) =
        attn@v0 + bv[i]; tanh arg gets + bv[i]/2 via a broadcast tile.
      * softmax normalization + sigmoid run as ONE activation per
        (io,hb): ob = tanh(po * rs + bv[i]/2) with scale=rs=0.5/rowsum
        (rowsum via vt's 2.0-column), then a Pool *0.5+0.5 affine per
        chunk. No separate normalize pass, no batched-tanh staging.
  - phase-B streams (k chunks, v readbacks) prefetch 2-3 deep; the k
    conv output ring is 4 deep so the scores/attnv pipeline doesn't
    stall at chunk boundaries. DMA issue spread over SP (loads + out
    stores), ACT (v_scr stores, kc), Pool/SWDGE (v readbacks, weights).
  => per-core HBM traffic ~134MB (inputs f16 84MB, v round-trip bf16
    33.5MB, out f16 16.8MB); phase 1 runs at the DMA roofline, phase B
    is PE/ACT-bound with contiguous matmul streams.
"""

import numpy as np

import concourse.bass as bass
import concourse.tile as tile
from concourse import bacc, mybir
from concourse.bass_utils import run_bass_kernel_spmd

C = 256
HALF = 128          # output channels per core
N_CORES = 8
ICHUNK = 4          # i values per phase-B chunk
A_CHUNK = 1024      # flattened spatial elems per phase-1 v chunk
QCH = 4             # h rows per phase-1 q chunk

_CACHE = {}


def build_nc(repeat=1):
    key = ("nc", repeat)
    if key in _CACHE:
        return _CACHE[key]
    f32 = mybir.dt.float32
    f16 = mybir.dt.float16
    bf16 = mybir.dt.bfloat16

    nc = bacc.Bacc("TRN2", target_bir_lowering=False, debug=False,
                   num_devices=N_CORES)

    query_b = nc.dram_tensor("query_b", [C, C, C], f16, kind="ExternalInput").ap()
    key_h = nc.dram_tensor("key_h", [C, HALF, C], f16, kind="ExternalInput").ap()
    value_b = nc.dram_tensor("value_b", [C, C, C], f16, kind="ExternalInput").ap()
    wqT = nc.dram_tensor("wqT", [C, HALF], f16, kind="ExternalInput").ap()
    wkT = nc.dram_tensor("wkT", [C, C], f16, kind="ExternalInput").ap()
    wvT = nc.dram_tensor("wvT", [C, HALF], f16, kind="ExternalInput").ap()
    bq_bc = nc.dram_tensor("bq_bc", [128, HALF], f16, kind="ExternalInput").ap()
    bvh_bc = nc.dram_tensor("bvh_bc", [128, HALF], f32, kind="ExternalInput").ap()
    out_b = nc.dram_tensor("out_b", [HALF, C, C], f16, kind="ExternalOutput").ap()

    v_scr = nc.dram_tensor("v_scr", [HALF, C, C], bf16).ap()

    HJ = C * C  # 65536
    KCH = ICHUNK * C  # flattened (il, w) per phase-B chunk

    qv_in = query_b.rearrange("(cb c) h j -> c cb h j", c=128)
    vv_in = value_b.rearrange("(cb c) j w -> c cb (j w)", c=128)
    kv_in = key_h.rearrange("(cb c) il w -> c cb (il w)", c=128)
    v_flat = v_scr.rearrange("i j w -> i (j w)")

    Exp = mybir.ActivationFunctionType.Exp
    Tanh = mybir.ActivationFunctionType.Tanh

    with tile.TileContext(nc) as tc:
        with (
            tc.tile_pool(name="weights", bufs=1) as wpool,
            tc.tile_pool(name="v_in", bufs=3) as v_in_pool,
            tc.tile_pool(name="v_out", bufs=2) as v_out_pool,
            tc.tile_pool(name="q_in", bufs=2) as q_in_pool,
            tc.tile_pool(name="ps512", bufs=2, space="PSUM") as ps512,
            tc.tile_pool(name="b_kin", bufs=3) as b_kin,
            tc.tile_pool(name="b_ksb", bufs=4) as b_ksb,
            tc.tile_pool(name="b_vt", bufs=3) as b_vt,
            tc.tile_pool(name="b_et", bufs=2) as b_et,
            tc.tile_pool(name="b_ob", bufs=2) as b_ob,
            tc.tile_pool(name="b_rs", bufs=8) as b_rs,
            tc.tile_pool(name="b_eb", bufs=8) as b_eb,
            tc.tile_pool(name="b_psc", bufs=2, space="PSUM") as b_psc,
            tc.tile_pool(name="b_po", bufs=2, space="PSUM") as b_po,
        ):
            wq_sb = wpool.tile([128, 2, HALF], f16)
            wk_sb = wpool.tile([128, 2, C], f16)
            wv_sb = wpool.tile([128, 2, HALF], f16)
            nc.gpsimd.dma_start(out=wq_sb, in_=wqT.rearrange("(cb c) i -> c cb i", c=128))
            nc.gpsimd.dma_start(out=wk_sb, in_=wkT.rearrange("(cb c) j -> c cb j", c=128))
            nc.gpsimd.dma_start(out=wv_sb, in_=wvT.rearrange("(cb c) i -> c cb i", c=128))
            bqbc_sb = wpool.tile([128, HALF], f16)
            bvbc_sb = wpool.tile([128, HALF], f32)
            nc.gpsimd.dma_start(out=bqbc_sb, in_=bq_bc)
            nc.gpsimd.dma_start(out=bvbc_sb, in_=bvh_bc)
            # SBUF-resident transposed q, h last (contiguous scores rhs):
            # qT[j, jb, i, h] = q0[i, h, jb*128+j]; column h=256 = bq[i]
            qT = wpool.tile([128, 2, HALF, C + 1], f16)

            def body(_it=None):
                # ---- Phase 1: v conv + direct-transposed q conv ----
                def v_chunk(t):
                    sl = slice(t * A_CHUNK, (t + 1) * A_CHUNK)
                    vc = v_in_pool.tile([128, 2, A_CHUNK], f16, tag="vc")
                    nc.sync.dma_start(out=vc, in_=vv_in[:, :, sl])
                    vs = v_out_pool.tile([128, A_CHUNK], bf16, tag="vs")
                    for n in range(A_CHUNK // 512):
                        ps = ps512.tile([128, 512], f32, tag="ps")
                        for cb in range(2):
                            nc.tensor.matmul(ps, wv_sb[:, cb, :],
                                             vc[:, cb, n * 512:(n + 1) * 512],
                                             start=(cb == 0), stop=(cb == 1))
                        if n == 0:
                            nc.vector.tensor_copy(out=vs[:, 0:512], in_=ps)
                        else:
                            nc.scalar.copy(out=vs[:, 512:1024], in_=ps)
                    nc.sync.dma_start(out=v_flat[:, sl], in_=vs)

                def q_chunk(t):
                    qc = q_in_pool.tile([128, 2, QCH, C], f16, tag="qc")
                    nc.sync.dma_start(out=qc, in_=qv_in[:, :, t * QCH:(t + 1) * QCH, :])
                    for hl in range(QCH):
                        h = t * QCH + hl
                        pq = ps512.tile([128, 2, 128], f32, tag="pq")
                        for jb in range(2):
                            for cb in range(2):
                                nc.tensor.matmul(
                                    pq[:, jb, :],
                                    qc[:, cb, hl, jb * 128:(jb + 1) * 128],
                                    wq_sb[:, cb, :],
                                    start=(cb == 0), stop=(cb == 1))
                        # strided h-column drain (the transpose tax lives
                        # here, under phase 1's DMA bound), alternating
                        # engines so consecutive h's drain concurrently
                        if hl % 2 == 0:
                            nc.vector.tensor_copy(out=qT[:, :, :, h], in_=pq)
                        else:
                            nc.scalar.copy(out=qT[:, :, :, h], in_=pq)

                for t in range(C // QCH):
                    v_chunk(t)
                    q_chunk(t)
                # bias column: qT[:, jb, i, 256] = bq[i]
                nc.vector.tensor_copy(out=qT[:, 0, :, C], in_=bqbc_sb)
                nc.vector.tensor_copy(out=qT[:, 1, :, C], in_=bqbc_sb)

                # ---- Phase B: k conv + attention ----
                def kc_load(ic):
                    kc = b_kin.tile([128, 2, KCH], f16, tag="kc")
                    nc.sync.dma_start(
                        out=kc, in_=kv_in[:, :, ic * KCH:(ic + 1) * KCH])
                    return kc

                def vt_load(ic):
                    i0 = ic * ICHUNK
                    vt = b_vt.tile([128, ICHUNK, 2, C + 8], bf16, tag="vt")
                    nc.gpsimd.dma_start(
                        out=vt[:, :, :, 0:C],
                        in_=v_scr[i0:i0 + ICHUNK].rearrange(
                            "io (jb j) w -> j io jb w", j=128))
                    nc.gpsimd.memset(vt[:, :, :, C:C + 1], 2.0)
                    return vt

                def k_conv(kc):
                    ksb = b_ksb.tile([128, 2, ICHUNK, C], f16, tag="ksb")
                    for jb in range(2):
                        for n in range(KCH // 512):
                            ps = ps512.tile([128, 512], f32, tag="ps")
                            for cb in range(2):
                                nc.tensor.matmul(
                                    ps, wk_sb[:, cb, jb * 128:(jb + 1) * 128],
                                    kc[:, cb, n * 512:(n + 1) * 512],
                                    start=(cb == 0), stop=(cb == 1))
                            nc.vector.tensor_copy(
                                out=ksb[:, jb, n * 2:(n + 1) * 2, :], in_=ps)
                    return ksb

                NCH = HALF // ICHUNK
                kc_pend = {0: kc_load(0), 1: kc_load(1)}
                vt_pend = {0: vt_load(0), 1: vt_load(1)}
                for ic in range(NCH):
                    i0 = ic * ICHUNK
                    if ic + 2 < NCH:
                        kc_pend[ic + 2] = kc_load(ic + 2)
                    vt = vt_pend.pop(ic)
                    if ic + 2 < NCH:
                        vt_pend[ic + 2] = vt_load(ic + 2)
                    ksb = k_conv(kc_pend.pop(ic))
                    ob = b_ob.tile([128, ICHUNK, 2, C], f16, tag="ob")
                    for io in range(ICHUNK):
                        i_loc = i0 + io
                        et = b_et.tile([128, 2, C], bf16, tag="et")
                        for wb in range(2):
                            psc = b_psc.tile([128, C + 1], f32, tag="psc")
                            for jb in range(2):
                                nc.tensor.matmul(
                                    psc,
                                    ksb[:, jb, io, wb * 128:(wb + 1) * 128],
                                    qT[:, jb, i_loc, :],
                                    start=(jb == 0), stop=(jb == 1))
                            # col 256 = bq[i]*colsum_k0[i, w-block]: the
                            # per-partition exp bias, bounced to SBUF
                            eb = b_eb.tile([128, 1], f32, tag="eb")
                            nc.vector.tensor_copy(out=eb, in_=psc[:, C:C + 1])
                            nc.scalar.activation(
                                out=et[:, wb, :], in_=psc[:, 0:C],
                                func=Exp, bias=eb)
                        for hb in range(2):
                            po = b_po.tile([128, C + 1], f32, tag="po")
                            for wb in range(2):
                                nc.tensor.matmul(
                                    po, et[:, wb, hb * 128:(hb + 1) * 128],
                                    vt[:, io, wb, 0:C + 1],
                                    start=(wb == 0), stop=(wb == 1))
                            rs = b_rs.tile([128, 1], f32, tag="rs")
                            nc.vector.reciprocal(out=rs, in_=po[:, C:C + 1])
                            # sigmoid(y) = 0.5*tanh(y/2)+0.5; y/2 = po*rs +
                            # bv[i]/2 (rs = 0.5/rowsum via vt's 2.0-column)
                            nc.scalar.activation(
                                out=ob[:, io, hb, :], in_=po[:, 0:C],
                                func=Tanh, scale=rs,
                                bias=bvbc_sb[:, i_loc:i_loc + 1])
                    nc.gpsimd.tensor_scalar(
                        out=ob, in0=ob,
                        scalar1=0.5, scalar2=0.5,
                        op0=mybir.AluOpType.mult,
                        op1=mybir.AluOpType.add)
                    nc.sync.dma_start(
                        out=out_b[i0:i0 + ICHUNK].rearrange(
                            "io (hb h) w -> h io hb w", h=128),
                        in_=ob)

            if repeat == 1:
                body()
            else:
                with tc.For_i(0, repeat, 1) as it:
                    body(it)

    nc.compile()
    _CACHE[key] = nc
    return nc


def make_in_maps(inputs):
    query = np.asarray(inputs["query"], dtype=np.float32)
    key_in = np.asarray(inputs["key_in"], dtype=np.float32)
    value = np.asarray(inputs["value"], dtype=np.float32)
    Wq = np.asarray(inputs["Wq"], dtype=np.float32)
    Wk = np.asarray(inputs["Wk"], dtype=np.float32)
    Wv = np.asarray(inputs["Wv"], dtype=np.float32)
    bq = np.asarray(inputs["bq"], dtype=np.float32)
    bv = np.asarray(inputs["bv"], dtype=np.float32)
    in_maps = []
    for core in range(N_CORES):
        b, g = core // 2, core % 2
        sl = slice(g * HALF, (g + 1) * HALF)
        in_maps.append({
            "query_b": np.ascontiguousarray(query[b], dtype=np.float16),
            "key_h": np.ascontiguousarray(key_in[b][:, sl, :], dtype=np.float16),
            "value_b": np.ascontiguousarray(value[b], dtype=np.float16),
            "wqT": np.ascontiguousarray(Wq[sl, :].T, dtype=np.float16),
            "wkT": np.ascontiguousarray(Wk.T, dtype=np.float16),
            "wvT": np.ascontiguousarray(Wv[sl, :].T, dtype=np.float16),
            "bq_bc": np.ascontiguousarray(
                np.broadcast_to(bq[sl].astype(np.float16), (128, HALF))),
            "bvh_bc": np.ascontiguousarray(
                np.broadcast_to((bv[sl] / 2).astype(np.float32), (128, HALF))),
        })
    return in_maps


def kernel(query, key_in, value, Wq, bq, Wk, bk, Wv, bv):
    nc = build_nc()
    in_maps = make_in_maps(dict(query=query, key_in=key_in, value=value,
                                Wq=Wq, bq=bq, Wk=Wk, bk=bk, Wv=Wv, bv=bv))
    res = run_bass_kernel_spmd(nc, in_maps, core_ids=list(range(N_CORES)))
    out = np.empty((4, C, C, C), dtype=np.float32)
    for core in range(N_CORES):
        b, g = core // 2, core % 2
        out[b, g * HALF:(g + 1) * HALF] = res.results[core]["out_b"].astype(np.float32)
    return out
